# revision 42
# baseline (speedup 1.0000x reference)
"""Trainium2 Bass kernel for nn_Attention_46420006535531 (v5).

Gated multi-head attention with additive attention bias:
    q = x@Wq, (k, v) = split(x@Wkv), heads=8, dim_head=64
    attn = softmax(q*k^T*scale + bias); out = attn@v
    out = (out * sigmoid(x@Wg + bg)) @ Wo + bo

Sharding: 8 cores; core c handles batch b=c//2 and the 4 heads
4*(c%2)..4*(c%2)+3.  Each core computes a partial y (its heads' slice
of Wo rows); the host sums the two partials per batch and adds bo.

Layout notes (all on-core data transposed, fp16 pipeline). HW-measured
op costs that drove the design are in work/micro.py:
 - pair-steps: step (ib, pair) runs BOTH heads of a head pair; their QK
   matmuls (K=64) are emitted adjacently with operands at partition
   bases 0/64, so the PE executes them concurrently in different row
   groups (HW: 96ns/MM row-alternated vs 420ns same-row-group).
 - S^T[j,i] per (head, j-pair) in [128,1024] PSUM tiles; ACT exps 1024
   elements per instruction (HW ACT = ~1ns/elem + ~440ns fixed - ACT is
   the bottleneck engine, so tanh is packed [128,512] and everything
   else is kept off ACT).
 - bias enters as exp(bias)^T fp16 (host-prepped); attention weights
   are exp(S)*exp(bias) fp16 muls - DVE except the jp1 tiles on Pool
   (Pool is ~4x slower; LK_JJ makes its tiles the last chain links).
 - AV: ONE accumulation chain per head ([65,512] PSUM; ones column
   rides row 64 of the v tiles for the softmax denominator); links of
   the two heads' chains alternate so same-chain accumulates stay ~2+
   PE instructions apart (PSUM RMW turnaround: HW 245ns/link).
 - the tail reads the chain PSUM directly: reciprocal of row 64, gate
   stt (tanh+1)*chain, then og = t1 * (0.5/denom broadcast via a
   [1,64]x[1,512] matmul); the 0.5 folds sigmoid(z)=0.5+0.5tanh(z/2)
   so gates share the Exp ACT table - no per-iteration table reloads.
 - projections are emission-interleaved into the first pair-steps;
   out-proj packs two heads along 128 partitions (full PE rows).
 - y partials leave as fp16; all DMA goes through sync/HWDGE.
 - timing uses For_i(staggered_reset=True): the default reset block
   puts an all-engine barrier on the loop back-edge.
"""
import sys
import numpy as np

for _p in ("/opt/trn_rl_repo",):
    if _p not in sys.path:
        sys.path.insert(0, _p)

import concourse.bass as bass
import concourse.bacc as bacc
import concourse.tile as tile
from concourse import mybir
from concourse.bass_utils import run_bass_kernel_spmd

B, N, DIM = 4, 1024, 256
HEADS, DIM_HEAD, INNER = 8, 64, 512
HPC = 4                      # heads per core
NCORES = 8
SCALE = DIM_HEAD ** -0.5     # folded into Wq on the host

F32 = mybir.dt.float32
FP16 = mybir.dt.float16
AF = mybir.ActivationFunctionType
ALU = mybir.AluOpType

NB = N // 512                # 2 i-blocks of 512
NJP = N // 128               # 8 j partition tiles
KK = DIM // 128              # 2 k-tiles for the projections


def _build_program(reps=1, loop_iters=0, static_bias=False, staggered=False):
    nc = bacc.Bacc(None, target_bir_lowering=False)

    # ---- DRAM I/O (per core) ----
    xt_d = nc.dram_tensor("xt", [128, KK, N], FP16, kind="ExternalInput")
    bias_d = nc.dram_tensor("bias_t", [HPC, NB, 128, NJP * 512], FP16,
                            kind="ExternalInput")
    wq_d = nc.dram_tensor("wq", [128, KK, 256], FP16, kind="ExternalInput")
    wk_d = nc.dram_tensor("wk", [128, KK, 256], FP16, kind="ExternalInput")
    wv_d = nc.dram_tensor("wv", [128, KK, 256], FP16, kind="ExternalInput")
    wg_d = nc.dram_tensor("wg", [128, KK, 256], FP16, kind="ExternalInput")
    bgh_d = nc.dram_tensor("bgh", [128, 2], F32, kind="ExternalInput")
    wo_d = nc.dram_tensor("wo", [2, 128, 256], FP16, kind="ExternalInput")
    hv_d = nc.dram_tensor("halves64", [1, 64], FP16, kind="ExternalInput")
    y_d = nc.dram_tensor("y", [N, 256], FP16, kind="ExternalOutput")

    with tile.TileContext(nc) as tc:
        import contextlib
        with contextlib.ExitStack() as ctx:
            const = ctx.enter_context(tc.tile_pool(name="const", bufs=1))
            acts = ctx.enter_context(tc.tile_pool(name="acts", bufs=2))
            biasp = ctx.enter_context(tc.tile_pool(name="biasp", bufs=6))
            pexp = ctx.enter_context(tc.tile_pool(name="pexp", bufs=6))
            pmul = ctx.enter_context(tc.tile_pool(name="pmul", bufs=16))
            small = ctx.enter_context(tc.tile_pool(name="small", bufs=4))
            ps_big = ctx.enter_context(tc.tile_pool(name="ps_big", bufs=2, space="PSUM"))
            ps_o = ctx.enter_context(tc.tile_pool(name="ps_o", bufs=3, space="PSUM"))
            ps_m = ctx.enter_context(tc.tile_pool(name="ps_m", bufs=1, space="PSUM"))

            # ---- constants / weights into SBUF (once) ----
            halves = const.tile([1, 64], FP16, tag="halves64")
            nc.sync.dma_start(out=halves[:], in_=hv_d[:])
            bgh_sb = const.tile([128, 2], F32, tag="bgh")
            nc.sync.dma_start(out=bgh_sb[:], in_=bgh_d[:])

            wq_sb = const.tile([128, KK, 256], FP16, tag="wq")
            nc.sync.dma_start(out=wq_sb[:], in_=wq_d[:])
            wk_sb = const.tile([128, KK, 256], FP16, tag="wk")
            nc.sync.dma_start(out=wk_sb[:], in_=wk_d[:])
            wv_sb = const.tile([128, KK, 256], FP16, tag="wv")
            nc.sync.dma_start(out=wv_sb[:], in_=wv_d[:])
            wg_sb = const.tile([128, KK, 256], FP16, tag="wg")
            nc.sync.dma_start(out=wg_sb[:], in_=wg_d[:])
            wo_sb = []
            for p in range(2):
                t = const.tile([128, 256], FP16, tag=f"wo{p}")
                nc.sync.dma_start(out=t[:], in_=wo_d[p])
                wo_sb.append(t)
            xt_sb = const.tile([128, KK, N], FP16, tag="xt")
            nc.sync.dma_start(out=xt_sb[:], in_=xt_d[:])

            static_bt = None
            if static_bias:
                static_bt = []
                for st in range(NB * HPC):
                    ib, h = st // HPC, st % HPC
                    sbt = const.tile([128, NJP, 512], FP16, tag=f"sbt{st}")
                    nc.sync.dma_start(
                        out=sbt[:],
                        in_=bias_d[h, ib].rearrange("p (j n) -> p j n", j=NJP))
                    static_bt.append(sbt)

            env = dict(locals())
            lp = nc.allow_low_precision(reason="fp16 attention pipeline")
            lp.__enter__()

            if loop_iters:
                with tc.For_i(0, loop_iters, 1, staggered_reset=staggered):
                    _emit_body(nc, tc, env)
            else:
                for _rep in range(reps):
                    _emit_body(nc, tc, env)

            lp.__exit__(None, None, None)

    nc.compile()
    return nc


def _emit_body(nc, tc, env):
    acts = env["acts"]; biasp = env["biasp"]
    pexp = env["pexp"]; pmul = env["pmul"]; small = env["small"]
    ps_big = env["ps_big"]; ps_o = env["ps_o"]; ps_m = env["ps_m"]
    halves = env["halves"]; bgh_sb = env["bgh_sb"]
    wq_sb = env["wq_sb"]; wk_sb = env["wk_sb"]; wv_sb = env["wv_sb"]
    wg_sb = env["wg_sb"]; wo_sb = env["wo_sb"]; xt_sb = env["xt_sb"]
    bias_d = env["bias_d"]; y_d = env["y_d"]

    # jp1's exp(bias) mul goes to Pool (the slow engine); its j-tiles are
    # consumed last in the AV chain via LK_JJ.  Other j-pairs mul on DVE.
    LK_JJ = (0, 1, 4, 5, 6, 7, 2, 3)

    # ---- projection helpers (emission interleaved into the steps) ----
    qT, kT = [], []
    for p in range(2):
        qt = acts.tile([128, N], FP16, tag=f"qT{p}")
        kt = acts.tile([128, N], FP16, tag=f"kT{p}")
        qT.append(qt)
        kT.append(kt)

    th2 = [[None] * NB for _ in range(2)]

    def emit_psg(p, ib, part):
        """part 0: matmuls; part 1: one packed [128,512] tanh instr."""
        if part == 0:
            psg = ps_m.tile([128, 512], F32, tag="misc")
            for kk in range(KK):
                nc.tensor.matmul(
                    psg[:], lhsT=wg_sb[:, kk, 128 * p:128 * p + 128],
                    rhs=xt_sb[:, kk, 512 * ib:512 * ib + 512],
                    start=(kk == 0), stop=(kk == KK - 1))
            emit_psg.ps[(p, ib)] = psg
        else:
            psg = emit_psg.ps[(p, ib)]
            gt = acts.tile([128, 512], FP16, tag=f"th{p}{ib}")
            nc.scalar.activation(
                gt[:], psg[:], AF.Tanh,
                bias=bgh_sb[:, p:p + 1], scale=0.5)
            th2[p][ib] = gt
    emit_psg.ps = {}

    # qk pair-1 emission pieces: 8 matmuls into 4 ps_m tiles + 4 copies
    def qk1_mm(idx):
        # idx 0..7: (q/k, ib, kk) = (idx//4, (idx//2)%2, idx%2)
        qk, ib, kk = idx // 4, (idx // 2) % 2, idx % 2
        w = wq_sb if qk == 0 else wk_sb
        if kk == 0:
            t = ps_m.tile([128, 512], F32, tag="misc")
            qk1_mm.ps[(qk, ib)] = t
        nc.tensor.matmul(
            qk1_mm.ps[(qk, ib)][:], lhsT=w[:, kk, 128:256],
            rhs=xt_sb[:, kk, 512 * ib:512 * ib + 512],
            start=(kk == 0), stop=(kk == KK - 1))
        if kk == KK - 1:
            dst = qT[1] if qk == 0 else kT[1]
            nc.vector.tensor_copy(dst[:, 512 * ib:512 * ib + 512],
                                  qk1_mm.ps[(qk, ib)][:])
    qk1_mm.ps = {}

    # ---- preamble: v projections + qk pair-0, interleaved ----
    vaug = []
    for jp in range(NJP):
        vt = acts.tile([128, HPC, 65], FP16, tag=f"vaug{jp}")
        vaug.append(vt)
        nc.gpsimd.memset(vt[:, :, 64], 1.0)
    psv = [None] * NJP
    psqk0 = [None] * NB
    for jph in range(4):            # pairs of jp
        for jp in (2 * jph, 2 * jph + 1):
            if jp % 2 == 0:
                t = ps_m.tile([128, 256], F32, tag="misc")
            else:
                t = ps_o.tile([128, 256], F32, tag="po")
            psv[jp] = t
            for kk in range(KK):
                nc.tensor.matmul(
                    t[:], lhsT=xt_sb[:, kk, 128 * jp:128 * jp + 128],
                    rhs=wv_sb[:, kk, :],
                    start=(kk == 0), stop=(kk == KK - 1))
        # two qk-pair0 matmuls per jp-pair: jph indexes (q/k, ib)
        qk, ib = jph // 2, jph % 2
        if qk == 0 and ib == 0:
            for b2 in range(NB):
                t = ps_big.tile([128, 1024], F32, tag="big")
                psqk0[b2] = t
        for kk in range(KK):
            w = wq_sb if qk == 0 else wk_sb
            nc.tensor.matmul(
                psqk0[ib][:, 512 * qk:512 * qk + 512],
                lhsT=w[:, kk, 0:128],
                rhs=xt_sb[:, kk, 512 * ib:512 * ib + 512],
                start=(kk == 0), stop=(kk == KK - 1))
        for jp in (2 * jph, 2 * jph + 1):
            nc.vector.tensor_copy(
                vaug[jp][:, :, 0:64],
                psv[jp][:].rearrange("p (h d) -> p h d", h=HPC))
        if jph == 1:                # q done for both ib
            for b2 in range(NB):
                nc.vector.tensor_copy(qT[0][:, 512 * b2:512 * b2 + 512],
                                      psqk0[b2][:, 0:512])
        if jph == 3:                # k done for both ib
            for b2 in range(NB):
                nc.vector.tensor_copy(kT[0][:, 512 * b2:512 * b2 + 512],
                                      psqk0[b2][:, 512:1024])

    # interleave schedule: step -> slot -> list of emission thunks
    inter = {0: {j: [lambda j=j: qk1_mm(j)] for j in range(NJP)},
             1: {0: [lambda: emit_psg(0, 0, 0)],
                 1: [lambda: emit_psg(0, 0, 1)],
                 2: [lambda: emit_psg(1, 0, 0)],
                 3: [lambda: emit_psg(1, 0, 1)]},
             2: {0: [lambda: emit_psg(0, 1, 0)],
                 1: [lambda: emit_psg(0, 1, 1)]},
             3: {0: [lambda: emit_psg(1, 1, 0)],
                 1: [lambda: emit_psg(1, 1, 1)]}}

    # ---- attention pair-steps ----
    # Step s = (ib, pair): both heads of the pair run together.  Their QK
    # matmuls are emitted adjacently with lhsT/rhs at partition bases 0/64,
    # so the PE runs them concurrently in different row groups (~2-4x).
    # One AV accumulation chain per head; same-chain links are ~3 PE
    # instructions apart so the PSUM accumulate turnaround stays hidden.
    steps = [(ib, pr) for ib in range(NB) for pr in range(2)]
    og_tiles = {}
    prev = None

    def emit_tail(st, c):
        ib, pr = st["ib"], st["pr"]
        h = 2 * pr + c
        ch = st["chain"][c]
        # recip and the gate stt read the chain PSUM directly (one PSUM
        # operand each is legal; the both-SB base-partition rule no longer
        # applies to the stt, so the tanh tiles can stay packed).
        r = small.tile([1, 512], FP16, tag="recip")
        nc.vector.reciprocal(r[:], ch[64:65, :])
        pR = ps_m.tile([64, 512], F32, tag="misc")
        nc.tensor.matmul(pR[:], lhsT=halves[:], rhs=r[:],
                         start=True, stop=True)
        t1 = small.tile([64, 512], FP16, tag="t1")
        nc.vector.scalar_tensor_tensor(
            t1[:], th2[pr][ib][64 * c:64 * c + 64, :], 1.0,
            ch[0:64, :], ALU.add, ALU.mult)
        key = (ib, pr)
        if key not in og_tiles:
            og = acts.tile([128, 512], FP16, tag=f"og{ib}{pr}")
            og_tiles[key] = og
        og = og_tiles[key]
        nc.vector.tensor_mul(og[64 * c:64 * c + 64, :], t1[:], pR[:])
        if pr == 1 and c == 1:
            # out-proj: psy tiles from ps_big (free between exp batches);
            # RMW partner (pr=1) spaced 2+ from its pr=0 matmul.
            for icg in range(2):          # groups of 2 ic
                psys = []
                for ic in (2 * icg, 2 * icg + 1):
                    psy = ps_big.tile([128, 256], F32, tag="big")
                    psys.append(psy)
                    nc.tensor.matmul(
                        psy[:],
                        lhsT=og_tiles[(ib, 0)][:, 128 * ic:128 * ic + 128],
                        rhs=wo_sb[0][:], start=True, stop=False)
                for k, ic in enumerate((2 * icg, 2 * icg + 1)):
                    nc.tensor.matmul(
                        psys[k][:],
                        lhsT=og_tiles[(ib, 1)][:, 128 * ic:128 * ic + 128],
                        rhs=wo_sb[1][:], start=False, stop=True)
                for k, ic in enumerate((2 * icg, 2 * icg + 1)):
                    it = 4 * ib + ic
                    yt = small.tile([128, 256], FP16, tag="yt")
                    nc.vector.tensor_copy(yt[:], psys[k][:])
                    nc.sync.dma_start(out=y_d[128 * it:128 * it + 128, :],
                                      in_=yt[:])
            for pr in range(2):
                del og_tiles[(ib, pr)]

    nsteps = len(steps)
    for s in range(nsteps + 1):
        cur = None
        if s < nsteps:
            ib, pr = steps[s]
            bts, chains = [], []
            for c in range(2):
                if env.get("static_bt"):
                    bt = env["static_bt"][2 * s + c]
                else:
                    bt = biasp.tile([128, NJP, 512], FP16, tag="bias")
                    bsrc = bias_d[2 * pr + c, ib].rearrange(
                        "p (j n) -> p j n", j=NJP)
                    nc.sync.dma_start(out=bt[:], in_=bsrc[:])
                bts.append(bt)
                ch_t = ps_o.tile([65, 512], F32, tag="po")
                chains.append(ch_t)
            cur = {"ib": ib, "pr": pr, "chain": chains, "bts": bts,
                   "pts": [[], []], "ps": [None, None]}
        for j in range(NJP):
            if cur is not None:
                jp = j // 2
                for c in range(2):      # adjacent row-group-alternating QK
                    off = 64 * c
                    if j % 2 == 0:
                        ps_t = ps_big.tile([128, 1024], F32, tag="big")
                        cur["ps"][c] = ps_t
                    nc.tensor.matmul(
                        cur["ps"][c][:, 512 * (j % 2):512 * (j % 2) + 512],
                        lhsT=kT[pr][off:off + 64, 128 * j:128 * j + 128],
                        rhs=qT[pr][off:off + 64, 512 * ib:512 * ib + 512],
                        start=True, stop=True)
                if j % 2 == 1:
                    for c in range(2):
                        pe = pexp.tile([128, 2, 512], FP16, tag="pexp")
                        nc.scalar.activation(
                            pe[:].rearrange("p a n -> p (a n)"),
                            cur["ps"][c][:], AF.Exp)
                        ptp = pmul.tile([128, 2, 512], FP16, tag="pmul")
                        cur["pts"][c].append(ptp)
                        # all muls on DVE: Pool's 2711ns mul (vs 640 DVE)
                        # feeds the LAST chain links and so gates the tail
                        nc.vector.tensor_mul(ptp[:], pe[:],
                                             bts[c][:, j - 1:j + 1, :])
            for thunk in inter.get(s, {}).get(j, ()):
                thunk()
            if prev is not None:
                # AV: one chain per head, link j consumes jj=LK_JJ[j].
                # Each head's tail is emitted right after its final link so
                # the tail ops queue ahead of the next step's work.
                jj = LK_JJ[j]
                for c in range(2):
                    nc.tensor.matmul(
                        prev["chain"][c][:],
                        lhsT=vaug[jj][:, 2 * prev["pr"] + c, :],
                        rhs=prev["pts"][c][jj // 2][:, jj % 2, :],
                        start=(j == 0), stop=(j == NJP - 1))
                    if j == NJP - 1:
                        emit_tail(prev, c)
        prev = cur


_PROG = None


def _get_program():
    global _PROG
    if _PROG is None:
        _PROG = _build_program()
    return _PROG


def _prep_core_inputs(x, attn_bias, Wq, Wkv, Wo, Wg, bg, core):
    b, cp = core // 2, core % 2
    f16 = np.float16
    f32 = np.float32

    xt = np.ascontiguousarray(
        x[b].T.reshape(KK, 128, N).transpose(1, 0, 2)).astype(f16)

    hs = HPC * cp
    A = attn_bias[b, hs:hs + HPC]                      # [4, i, j]
    b5 = np.ascontiguousarray(
        A.reshape(HPC, NB, 512, NJP, 128).transpose(0, 1, 4, 3, 2)
    ).astype(f32, copy=False)                          # [h, ib, 128, NJP, 512]
    # j-tiles 0,1 (jp0) stay raw (added into S on PE); the rest ship exp'd
    # for the elementwise-multiply path.
    bias_t = np.exp(b5).reshape(HPC, NB, 128, NJP * 512).astype(f16)

    def wtile(w):   # [256, 256] -> [128, KK, 256] fp16
        return np.ascontiguousarray(
            w.reshape(KK, 128, 256).transpose(1, 0, 2)).astype(f16)

    wq_t = wtile(Wq[:, 256 * cp:256 * cp + 256] * SCALE)
    wk_t = wtile(Wkv[:, :INNER][:, 256 * cp:256 * cp + 256])
    wv_t = wtile(Wkv[:, INNER:][:, 256 * cp:256 * cp + 256])
    wg_t = wtile(Wg[:, 256 * cp:256 * cp + 256])

    g0 = 256 * cp
    bgh = np.zeros((128, 2), f32)
    for p in range(2):
        bgh[:, p] = 0.5 * bg[g0 + 128 * p:g0 + 128 * p + 128]
    wo_t = np.ascontiguousarray(
        Wo[g0:g0 + 256, :].reshape(2, 128, 256)).astype(f16)

    return {
        "xt": xt, "bias_t": bias_t, "wq": wq_t, "wk": wk_t, "wv": wv_t,
        "wg": wg_t, "bgh": bgh, "wo": wo_t,
        "halves64": np.full((1, 64), 0.5, f16),
    }


_LAST_RESULTS = None


def kernel(x, attn_bias, Wq, Wkv, Wo, bo, Wg, bg, _trace=False, **_trace_kw):
    global _LAST_RESULTS
    x = np.asarray(x, np.float32)
    attn_bias = np.asarray(attn_bias, np.float32)
    Wq = np.asarray(Wq, np.float32)
    Wkv = np.asarray(Wkv, np.float32)
    Wo = np.asarray(Wo, np.float32)
    bo = np.asarray(bo, np.float32)
    Wg = np.asarray(Wg, np.float32)
    bg = np.asarray(bg, np.float32)

    nc = _get_program()
    in_maps = [_prep_core_inputs(x, attn_bias, Wq, Wkv, Wo, Wg, bg, c)
               for c in range(NCORES)]
    res = run_bass_kernel_spmd(nc, in_maps, list(range(NCORES)),
                               trace=_trace, **_trace_kw)
    _LAST_RESULTS = res

    y = np.empty((B, N, DIM), np.float32)
    for b in range(B):
        y[b] = (res.results[2 * b]["y"].astype(np.float32)
                + res.results[2 * b + 1]["y"].astype(np.float32) + bo)
    return y



# revision 44
# speedup vs baseline: 1.0085x; 1.0085x over previous
"""Trainium2 Bass kernel for nn_Attention_46420006535531 (v5).

Gated multi-head attention with additive attention bias:
    q = x@Wq, (k, v) = split(x@Wkv), heads=8, dim_head=64
    attn = softmax(q*k^T*scale + bias); out = attn@v
    out = (out * sigmoid(x@Wg + bg)) @ Wo + bo

Sharding: 8 cores; core c handles batch b=c//2 and the 4 heads
4*(c%2)..4*(c%2)+3.  Each core computes a partial y (its heads' slice
of Wo rows); the host sums the two partials per batch and adds bo.

Layout notes (all on-core data transposed, fp16 pipeline). HW-measured
op costs that drove the design are in work/micro.py:
 - pair-steps: step (ib, pair) runs BOTH heads of a head pair; their QK
   matmuls (K=64) are emitted adjacently with operands at partition
   bases 0/64, so the PE executes them concurrently in different row
   groups (HW: 96ns/MM row-alternated vs 420ns same-row-group).
 - S^T[j,i] per (head, j-pair) in [128,1024] PSUM tiles; ACT exps 1024
   elements per instruction (HW ACT = ~1ns/elem + ~440ns fixed - ACT is
   the bottleneck engine, so tanh is packed [128,512] and everything
   else is kept off ACT).
 - bias enters as exp(bias)^T fp16 (host-prepped); attention weights
   are exp(S)*exp(bias) fp16 muls - DVE except the jp1 tiles on Pool
   (Pool is ~4x slower; LK_JJ makes its tiles the last chain links).
 - AV: ONE accumulation chain per head ([65,512] PSUM; ones column
   rides row 64 of the v tiles for the softmax denominator); links of
   the two heads' chains alternate so same-chain accumulates stay ~2+
   PE instructions apart (PSUM RMW turnaround: HW 245ns/link).
 - the tail reads the chain PSUM directly: reciprocal of row 64, gate
   stt (tanh+1)*chain, then og = t1 * (0.5/denom broadcast via a
   [1,64]x[1,512] matmul); the 0.5 folds sigmoid(z)=0.5+0.5tanh(z/2)
   so gates share the Exp ACT table - no per-iteration table reloads.
 - projections are emission-interleaved into the first pair-steps;
   out-proj packs two heads along 128 partitions (full PE rows).
 - y partials leave as fp16; all DMA goes through sync/HWDGE.
 - timing uses For_i(staggered_reset=True): the default reset block
   puts an all-engine barrier on the loop back-edge.
"""
import sys
import numpy as np

for _p in ("/opt/trn_rl_repo",):
    if _p not in sys.path:
        sys.path.insert(0, _p)

import concourse.bass as bass
import concourse.bacc as bacc
import concourse.tile as tile
from concourse import mybir
from concourse.bass_utils import run_bass_kernel_spmd

B, N, DIM = 4, 1024, 256
HEADS, DIM_HEAD, INNER = 8, 64, 512
HPC = 4                      # heads per core
NCORES = 8
SCALE = DIM_HEAD ** -0.5     # folded into Wq on the host

F32 = mybir.dt.float32
FP16 = mybir.dt.float16
AF = mybir.ActivationFunctionType
ALU = mybir.AluOpType

NB = N // 512                # 2 i-blocks of 512
NJP = N // 128               # 8 j partition tiles
KK = DIM // 128              # 2 k-tiles for the projections


def _build_program(reps=1, loop_iters=0, static_bias=False, staggered=False):
    nc = bacc.Bacc(None, target_bir_lowering=False)

    # ---- DRAM I/O (per core) ----
    xt_d = nc.dram_tensor("xt", [128, KK, N], FP16, kind="ExternalInput")
    bias_d = nc.dram_tensor("bias_t", [HPC, NB, 128, NJP * 512], FP16,
                            kind="ExternalInput")
    wq_d = nc.dram_tensor("wq", [128, KK, 256], FP16, kind="ExternalInput")
    wk_d = nc.dram_tensor("wk", [128, KK, 256], FP16, kind="ExternalInput")
    wv_d = nc.dram_tensor("wv", [128, KK, 256], FP16, kind="ExternalInput")
    wg_d = nc.dram_tensor("wg", [128, KK, 256], FP16, kind="ExternalInput")
    bgh_d = nc.dram_tensor("bgh", [128, 2], F32, kind="ExternalInput")
    wo_d = nc.dram_tensor("wo", [2, 128, 256], FP16, kind="ExternalInput")
    hv_d = nc.dram_tensor("halves64", [1, 64], FP16, kind="ExternalInput")
    y_d = nc.dram_tensor("y", [N, 256], FP16, kind="ExternalOutput")

    with tile.TileContext(nc) as tc:
        import contextlib
        with contextlib.ExitStack() as ctx:
            const = ctx.enter_context(tc.tile_pool(name="const", bufs=1))
            acts = ctx.enter_context(tc.tile_pool(name="acts", bufs=2))
            biasp = ctx.enter_context(tc.tile_pool(name="biasp", bufs=6))
            pexp = ctx.enter_context(tc.tile_pool(name="pexp", bufs=6))
            pmul = ctx.enter_context(tc.tile_pool(name="pmul", bufs=16))
            small = ctx.enter_context(tc.tile_pool(name="small", bufs=4))
            ps_big = ctx.enter_context(tc.tile_pool(name="ps_big", bufs=2, space="PSUM"))
            ps_o = ctx.enter_context(tc.tile_pool(name="ps_o", bufs=3, space="PSUM"))
            ps_m = ctx.enter_context(tc.tile_pool(name="ps_m", bufs=1, space="PSUM"))

            # ---- constants / weights into SBUF (once) ----
            halves = const.tile([1, 64], FP16, tag="halves64")
            nc.sync.dma_start(out=halves[:], in_=hv_d[:])
            bgh_sb = const.tile([128, 2], F32, tag="bgh")
            nc.sync.dma_start(out=bgh_sb[:], in_=bgh_d[:])

            wq_sb = const.tile([128, KK, 256], FP16, tag="wq")
            nc.sync.dma_start(out=wq_sb[:], in_=wq_d[:])
            wk_sb = const.tile([128, KK, 256], FP16, tag="wk")
            nc.sync.dma_start(out=wk_sb[:], in_=wk_d[:])
            wv_sb = const.tile([128, KK, 256], FP16, tag="wv")
            nc.sync.dma_start(out=wv_sb[:], in_=wv_d[:])
            wg_sb = const.tile([128, KK, 256], FP16, tag="wg")
            nc.sync.dma_start(out=wg_sb[:], in_=wg_d[:])
            wo_sb = []
            for p in range(2):
                t = const.tile([128, 256], FP16, tag=f"wo{p}")
                nc.sync.dma_start(out=t[:], in_=wo_d[p])
                wo_sb.append(t)
            xt_sb = const.tile([128, KK, N], FP16, tag="xt")
            nc.sync.dma_start(out=xt_sb[:], in_=xt_d[:])

            static_bt = None
            if static_bias:
                static_bt = []
                for st in range(NB * HPC):
                    ib, h = st // HPC, st % HPC
                    sbt = const.tile([128, NJP, 512], FP16, tag=f"sbt{st}")
                    nc.sync.dma_start(
                        out=sbt[:],
                        in_=bias_d[h, ib].rearrange("p (j n) -> p j n", j=NJP))
                    static_bt.append(sbt)

            env = dict(locals())
            lp = nc.allow_low_precision(reason="fp16 attention pipeline")
            lp.__enter__()

            if loop_iters:
                with tc.For_i(0, loop_iters, 1, staggered_reset=staggered):
                    _emit_body(nc, tc, env)
            else:
                for _rep in range(reps):
                    _emit_body(nc, tc, env)

            lp.__exit__(None, None, None)

    nc.compile()
    return nc


def _emit_body(nc, tc, env):
    acts = env["acts"]; biasp = env["biasp"]
    pexp = env["pexp"]; pmul = env["pmul"]; small = env["small"]
    ps_big = env["ps_big"]; ps_o = env["ps_o"]; ps_m = env["ps_m"]
    halves = env["halves"]; bgh_sb = env["bgh_sb"]
    wq_sb = env["wq_sb"]; wk_sb = env["wk_sb"]; wv_sb = env["wv_sb"]
    wg_sb = env["wg_sb"]; wo_sb = env["wo_sb"]; xt_sb = env["xt_sb"]
    bias_d = env["bias_d"]; y_d = env["y_d"]

    # jp1's exp(bias) mul goes to Pool (the slow engine); its j-tiles are
    # consumed last in the AV chain via LK_JJ.  Other j-pairs mul on DVE.
    LK_JJ = (0, 1, 4, 5, 6, 7, 2, 3)

    # ---- projection helpers (emission interleaved into the steps) ----
    qT, kT = [], []
    for p in range(2):
        qt = acts.tile([128, N], FP16, tag=f"qT{p}")
        kt = acts.tile([128, N], FP16, tag=f"kT{p}")
        qT.append(qt)
        kT.append(kt)

    th2 = [[None] * NB for _ in range(2)]

    def emit_psg(p, ib, part):
        """part 0: matmuls; part 1: one packed [128,512] tanh instr."""
        if part == 0:
            psg = ps_m.tile([128, 512], F32, tag="misc")
            for kk in range(KK):
                nc.tensor.matmul(
                    psg[:], lhsT=wg_sb[:, kk, 128 * p:128 * p + 128],
                    rhs=xt_sb[:, kk, 512 * ib:512 * ib + 512],
                    start=(kk == 0), stop=(kk == KK - 1))
            emit_psg.ps[(p, ib)] = psg
        else:
            psg = emit_psg.ps[(p, ib)]
            gt = acts.tile([128, 512], FP16, tag=f"th{p}{ib}")
            nc.scalar.activation(
                gt[:], psg[:], AF.Tanh,
                bias=bgh_sb[:, p:p + 1], scale=0.5)
            th2[p][ib] = gt
    emit_psg.ps = {}

    # qk pair-1 emission pieces: 8 matmuls into 4 ps_m tiles + 4 copies
    def qk1_mm(idx):
        # idx 0..7: (q/k, ib, kk) = (idx//4, (idx//2)%2, idx%2)
        qk, ib, kk = idx // 4, (idx // 2) % 2, idx % 2
        w = wq_sb if qk == 0 else wk_sb
        if kk == 0:
            t = ps_m.tile([128, 512], F32, tag="misc")
            qk1_mm.ps[(qk, ib)] = t
        nc.tensor.matmul(
            qk1_mm.ps[(qk, ib)][:], lhsT=w[:, kk, 128:256],
            rhs=xt_sb[:, kk, 512 * ib:512 * ib + 512],
            start=(kk == 0), stop=(kk == KK - 1))
        if kk == KK - 1:
            dst = qT[1] if qk == 0 else kT[1]
            nc.vector.tensor_copy(dst[:, 512 * ib:512 * ib + 512],
                                  qk1_mm.ps[(qk, ib)][:])
    qk1_mm.ps = {}

    # ---- preamble: v projections + qk pair-0, interleaved ----
    vaug = []
    for jp in range(NJP):
        vt = acts.tile([128, HPC, 65], FP16, tag=f"vaug{jp}")
        vaug.append(vt)
        nc.gpsimd.memset(vt[:, :, 64], 1.0)
    psv = [None] * NJP
    psqk0 = [None] * NB
    for jph in range(4):            # pairs of jp
        for jp in (2 * jph, 2 * jph + 1):
            if jp % 2 == 0:
                t = ps_m.tile([128, 256], F32, tag="misc")
            else:
                t = ps_o.tile([128, 256], F32, tag="po")
            psv[jp] = t
            for kk in range(KK):
                nc.tensor.matmul(
                    t[:], lhsT=xt_sb[:, kk, 128 * jp:128 * jp + 128],
                    rhs=wv_sb[:, kk, :],
                    start=(kk == 0), stop=(kk == KK - 1))
        # two qk-pair0 matmuls per jp-pair: jph indexes (q/k, ib)
        qk, ib = jph // 2, jph % 2
        if qk == 0 and ib == 0:
            for b2 in range(NB):
                t = ps_big.tile([128, 1024], F32, tag="big")
                psqk0[b2] = t
        for kk in range(KK):
            w = wq_sb if qk == 0 else wk_sb
            nc.tensor.matmul(
                psqk0[ib][:, 512 * qk:512 * qk + 512],
                lhsT=w[:, kk, 0:128],
                rhs=xt_sb[:, kk, 512 * ib:512 * ib + 512],
                start=(kk == 0), stop=(kk == KK - 1))
        for jp in (2 * jph, 2 * jph + 1):
            nc.vector.tensor_copy(
                vaug[jp][:, :, 0:64],
                psv[jp][:].rearrange("p (h d) -> p h d", h=HPC))
        if jph == 1:                # q done for both ib
            for b2 in range(NB):
                nc.vector.tensor_copy(qT[0][:, 512 * b2:512 * b2 + 512],
                                      psqk0[b2][:, 0:512])
        if jph == 3:                # k done for both ib
            for b2 in range(NB):
                nc.vector.tensor_copy(kT[0][:, 512 * b2:512 * b2 + 512],
                                      psqk0[b2][:, 512:1024])

    # interleave schedule: step -> slot -> list of emission thunks
    inter = {0: {j: [lambda j=j: qk1_mm(j)] for j in range(NJP)},
             1: {0: [lambda: emit_psg(0, 0, 0)],
                 1: [lambda: emit_psg(0, 0, 1)],
                 2: [lambda: emit_psg(1, 0, 0)],
                 3: [lambda: emit_psg(1, 0, 1)]},
             2: {0: [lambda: emit_psg(0, 1, 0)],
                 1: [lambda: emit_psg(0, 1, 1)]},
             3: {0: [lambda: emit_psg(1, 1, 0)],
                 1: [lambda: emit_psg(1, 1, 1)]}}

    # ---- attention pair-steps ----
    # Step s = (ib, pair): both heads of the pair run together.  Their QK
    # matmuls are emitted adjacently with lhsT/rhs at partition bases 0/64,
    # so the PE runs them concurrently in different row groups (~2-4x).
    # One AV accumulation chain per head; same-chain links are ~3 PE
    # instructions apart so the PSUM accumulate turnaround stays hidden.
    steps = [(ib, pr) for ib in range(NB) for pr in range(2)]
    og_tiles = {}
    prev = None

    def emit_tail(st, c):
        ib, pr = st["ib"], st["pr"]
        h = 2 * pr + c
        ch = st["chain"][c]
        # recip and the gate stt read the chain PSUM directly (one PSUM
        # operand each is legal; the both-SB base-partition rule no longer
        # applies to the stt, so the tanh tiles can stay packed).
        r = small.tile([1, 512], FP16, tag="recip")
        nc.vector.reciprocal(r[:], ch[64:65, :])
        pR = ps_m.tile([64, 512], F32, tag="misc")
        nc.tensor.matmul(pR[:], lhsT=halves[:], rhs=r[:],
                         start=True, stop=True)
        t1 = small.tile([64, 512], FP16, tag="t1")
        nc.vector.scalar_tensor_tensor(
            t1[:], th2[pr][ib][64 * c:64 * c + 64, :], 1.0,
            ch[0:64, :], ALU.add, ALU.mult)
        key = (ib, pr)
        if key not in og_tiles:
            og = acts.tile([128, 512], FP16, tag=f"og{ib}{pr}")
            og_tiles[key] = og
        og = og_tiles[key]
        nc.vector.tensor_mul(og[64 * c:64 * c + 64, :], t1[:], pR[:])
        if pr == 1 and c == 1:
            # out-proj: psy tiles from ps_big (free between exp batches);
            # RMW partner (pr=1) spaced 2+ from its pr=0 matmul.
            for icg in range(2):          # groups of 2 ic
                psys = []
                for ic in (2 * icg, 2 * icg + 1):
                    psy = ps_big.tile([128, 256], F32, tag="big")
                    psys.append(psy)
                    nc.tensor.matmul(
                        psy[:],
                        lhsT=og_tiles[(ib, 0)][:, 128 * ic:128 * ic + 128],
                        rhs=wo_sb[0][:], start=True, stop=False)
                for k, ic in enumerate((2 * icg, 2 * icg + 1)):
                    nc.tensor.matmul(
                        psys[k][:],
                        lhsT=og_tiles[(ib, 1)][:, 128 * ic:128 * ic + 128],
                        rhs=wo_sb[1][:], start=False, stop=True)
                for k, ic in enumerate((2 * icg, 2 * icg + 1)):
                    it = 4 * ib + ic
                    yt = small.tile([128, 256], FP16, tag="yt")
                    nc.vector.tensor_copy(yt[:], psys[k][:])
                    nc.sync.dma_start(out=y_d[128 * it:128 * it + 128, :],
                                      in_=yt[:])
            for pr in range(2):
                del og_tiles[(ib, pr)]

    nsteps = len(steps)
    for s in range(nsteps + 1):
        cur = None
        if s < nsteps:
            ib, pr = steps[s]
            bts, chains = [], []
            for c in range(2):
                if env.get("static_bt"):
                    bt = env["static_bt"][2 * s + c]
                else:
                    bt = biasp.tile([128, NJP, 512], FP16, tag="bias")
                    bsrc = bias_d[2 * pr + c, ib].rearrange(
                        "p (j n) -> p j n", j=NJP)
                    nc.sync.dma_start(out=bt[:], in_=bsrc[:])
                bts.append(bt)
                ch_t = ps_o.tile([65, 512], F32, tag="po")
                chains.append(ch_t)
            cur = {"ib": ib, "pr": pr, "chain": chains, "bts": bts,
                   "pts": [[], []], "ps": [None, None]}
        for j in range(NJP):
            if cur is not None:
                jp = j // 2
                for c in range(2):      # adjacent row-group-alternating QK
                    off = 64 * c
                    if j % 2 == 0:
                        ps_t = ps_big.tile([128, 1024], F32, tag="big")
                        cur["ps"][c] = ps_t
                    nc.tensor.matmul(
                        cur["ps"][c][:, 512 * (j % 2):512 * (j % 2) + 512],
                        lhsT=kT[pr][off:off + 64, 128 * j:128 * j + 128],
                        rhs=qT[pr][off:off + 64, 512 * ib:512 * ib + 512],
                        start=True, stop=True)
                if j % 2 == 1:
                    for c in range(2):
                        pe = pexp.tile([128, 2, 512], FP16, tag="pexp")
                        nc.scalar.activation(
                            pe[:].rearrange("p a n -> p (a n)"),
                            cur["ps"][c][:], AF.Exp)
                        ptp = pmul.tile([128, 2, 512], FP16, tag="pmul")
                        cur["pts"][c].append(ptp)
                        eng = nc.gpsimd if jp == 1 else nc.vector
                        eng.tensor_mul(ptp[:], pe[:], bts[c][:, j - 1:j + 1, :])
            for thunk in inter.get(s, {}).get(j, ()):
                thunk()
            if prev is not None:
                # AV: one chain per head, link j consumes jj=LK_JJ[j] so the
                # Pool-mul j-pair (jp1 -> jj 2,3) lands last.
                jj = LK_JJ[j]
                for c in range(2):
                    nc.tensor.matmul(
                        prev["chain"][c][:],
                        lhsT=vaug[jj][:, 2 * prev["pr"] + c, :],
                        rhs=prev["pts"][c][jj // 2][:, jj % 2, :],
                        start=(j == 0), stop=(j == NJP - 1))
        if prev is not None:
            emit_tail(prev, 0)
            emit_tail(prev, 1)
        prev = cur


_PROG = None


def _get_program():
    global _PROG
    if _PROG is None:
        _PROG = _build_program()
    return _PROG


def _prep_core_inputs(x, attn_bias, Wq, Wkv, Wo, Wg, bg, core):
    b, cp = core // 2, core % 2
    f16 = np.float16
    f32 = np.float32

    xt = np.ascontiguousarray(
        x[b].T.reshape(KK, 128, N).transpose(1, 0, 2)).astype(f16)

    hs = HPC * cp
    A = attn_bias[b, hs:hs + HPC]                      # [4, i, j]
    b5 = np.ascontiguousarray(
        A.reshape(HPC, NB, 512, NJP, 128).transpose(0, 1, 4, 3, 2)
    ).astype(f32, copy=False)                          # [h, ib, 128, NJP, 512]
    # j-tiles 0,1 (jp0) stay raw (added into S on PE); the rest ship exp'd
    # for the elementwise-multiply path.
    bias_t = np.exp(b5).reshape(HPC, NB, 128, NJP * 512).astype(f16)

    def wtile(w):   # [256, 256] -> [128, KK, 256] fp16
        return np.ascontiguousarray(
            w.reshape(KK, 128, 256).transpose(1, 0, 2)).astype(f16)

    wq_t = wtile(Wq[:, 256 * cp:256 * cp + 256] * SCALE)
    wk_t = wtile(Wkv[:, :INNER][:, 256 * cp:256 * cp + 256])
    wv_t = wtile(Wkv[:, INNER:][:, 256 * cp:256 * cp + 256])
    wg_t = wtile(Wg[:, 256 * cp:256 * cp + 256])

    g0 = 256 * cp
    bgh = np.zeros((128, 2), f32)
    for p in range(2):
        bgh[:, p] = 0.5 * bg[g0 + 128 * p:g0 + 128 * p + 128]
    wo_t = np.ascontiguousarray(
        Wo[g0:g0 + 256, :].reshape(2, 128, 256)).astype(f16)

    return {
        "xt": xt, "bias_t": bias_t, "wq": wq_t, "wk": wk_t, "wv": wv_t,
        "wg": wg_t, "bgh": bgh, "wo": wo_t,
        "halves64": np.full((1, 64), 0.5, f16),
    }


_LAST_RESULTS = None


def kernel(x, attn_bias, Wq, Wkv, Wo, bo, Wg, bg, _trace=False, **_trace_kw):
    global _LAST_RESULTS
    x = np.asarray(x, np.float32)
    attn_bias = np.asarray(attn_bias, np.float32)
    Wq = np.asarray(Wq, np.float32)
    Wkv = np.asarray(Wkv, np.float32)
    Wo = np.asarray(Wo, np.float32)
    bo = np.asarray(bo, np.float32)
    Wg = np.asarray(Wg, np.float32)
    bg = np.asarray(bg, np.float32)

    nc = _get_program()
    in_maps = [_prep_core_inputs(x, attn_bias, Wq, Wkv, Wo, Wg, bg, c)
               for c in range(NCORES)]
    res = run_bass_kernel_spmd(nc, in_maps, list(range(NCORES)),
                               trace=_trace, **_trace_kw)
    _LAST_RESULTS = res

    y = np.empty((B, N, DIM), np.float32)
    for b in range(B):
        y[b] = (res.results[2 * b]["y"].astype(np.float32)
                + res.results[2 * b + 1]["y"].astype(np.float32) + bo)
    return y

